# revision 30
# baseline (speedup 1.0000x reference)
"""DLRM forward (embedding_lookup) Trainium2 Bass kernel — v5.

Data-parallel over batch (4096/8 = 512 samples per core). Per core:
  - idx DMAs + indirect gathers issued first (weight DMAs hide under them);
    DVE pooling over L=4 -> es (pair-interleaved layout).
  - bottom MLP 512-wide; PE transposes (2 entities/instr, alternating PSUM
    banks) drained into PTflat[128, 512*27] (sample-major contiguous
    stationary APs; duplicated row bands so gram LDWEIGHTS overlap MMs).
  - grams in two 256-sample groups (4 PSUM banks each reused), per-sample
    tile-MMs on 4 tile_position col strips x alternating row bands
    (~34 ns/sample issue rate). Explicit fences around each group: PE-W +
    DVE-R of one PSUM bank is fatal and invisible to dataflow tracking.
  - tril extraction into PERMUTED zt column layout: zt col = 128*strip +
    64*tp + 32*tl + 16*b + w makes every copy's dst a contiguous 64-col
    2-level AP. Top MLP consumes zt + a permuted copy of x; the host
    un-permutes the output columns.
  - top MLP 512-wide; Sigmoid; store [1, 512] (permuted).
No collectives needed.
"""

import numpy as np
import ml_dtypes

B, T, L, NR, M = 4096, 26, 4, 100000, 64
E27 = T + 1                      # 27 entities: slots 0..25 = tables, 26 = x
NCORES = 8
BC = B // NCORES                 # 512 samples per core
TILE = 128
NT = BC // TILE                  # 4 gather tiles per core

_BF = ml_dtypes.bfloat16

_prog_cache = {}

ZPAD = 896
NZCH = ZPAD // 128  # 7


def _ztcol(c):
    """natural in-core sample index -> permuted zt column."""
    t, u = c // 128, c % 128
    tp, tl = t // 2, t % 2
    s, gi = u // 32, u % 32
    b, w = gi // 16, gi % 16
    return 128 * s + 64 * tp + 32 * tl + 16 * b + w


def build_program():
    import concourse.bass as bass
    import concourse.mybir as mybir
    import concourse.tile as tile
    from concourse import bacc
    from concourse.masks import make_identity
    from concourse.tile_rust import add_dep_helper
    from contextlib import ExitStack

    bf16 = mybir.dt.bfloat16
    f32 = mybir.dt.float32
    i32 = mybir.dt.int32
    Relu = mybir.ActivationFunctionType.Relu
    Sigmoid = mybir.ActivationFunctionType.Sigmoid

    nc = bacc.Bacc(
        "TRN2", target_bir_lowering=False, debug=False,
        num_devices=NCORES,
    )

    def din(name, shape, dt):
        return nc.dram_tensor(name, shape, dt, kind="ExternalInput").ap()

    # table as f32 container (bf16 pairs): the vector-indirect DMA path
    # quantizes index values through the transfer dtype — bf16 corrupts any
    # index > 256, f32 is exact below 2^24.
    table = din("table", [T * NR, M // 2], f32)
    xT = din("xT", [13, BC], bf16)
    idx = din("idx", [BC, T * L], i32)
    wb0 = din("wb0", [13, 512], bf16)       # bot W0^T
    wb1 = din("wb1", [128, 1024], bf16)     # bot W1^T k-chunk packed
    wb2 = din("wb2", [128, 128], bf16)      # bot W2^T k-chunk packed
    wt0x = din("wt0x", [64, 512], bf16)     # top W0[:, :64]^T
    wt0z = din("wt0z", [128, NZCH * 512], bf16)  # top W0[:, 64:]^T boxed
    wt1 = din("wt1", [128, 1024], bf16)     # top W1^T k-chunk packed
    wt2 = din("wt2", [128, 2], bf16)        # top W2^T k-chunk packed
    bb0 = din("bb0", [128, 4], f32)
    bb1 = din("bb1", [128, 2], f32)
    bb2 = din("bb2", [64, 1], f32)
    bt0 = din("bt0", [128, 4], f32)
    bt1 = din("bt1", [128, 2], f32)
    bt2 = din("bt2", [1, 1], f32)
    out = nc.dram_tensor("out", [1, BC], f32, kind="ExternalOutput").ap()

    with tile.TileContext(nc) as tc, ExitStack() as ctx:
        wpool = ctx.enter_context(tc.tile_pool(name="weights", bufs=1))
        ipool = ctx.enter_context(tc.tile_pool(name="idx", bufs=2))
        epool = ctx.enter_context(tc.tile_pool(name="emb", bufs=2))
        ppool = ctx.enter_context(tc.tile_pool(name="ptflat", bufs=1))
        hpool = ctx.enter_context(tc.tile_pool(name="acts", bufs=1))
        zpool = ctx.enter_context(tc.tile_pool(name="ztril", bufs=1))
        opool = ctx.enter_context(tc.tile_pool(name="outs", bufs=1))
        mmpool = ctx.enter_context(tc.tile_pool(name="mlp_psum", bufs=2, space="PSUM"))
        tppool = ctx.enter_context(tc.tile_pool(name="tp_psum", bufs=1, space="PSUM"))
        gpool = ctx.enter_context(tc.tile_pool(name="gram_psum", bufs=1, space="PSUM"))

        # --- idx + gathers first; weight DMAs hide under the gather ---
        # each it tile gets its own buffer: the indirect DMA's index read
        # (in_offset) must never see a reused buffer mid-gather
        its, es4s = [], []
        for t in range(NT):
            rows = slice(t * TILE, (t + 1) * TILE)
            it = wpool.tile([TILE, T * L], i32, tag=f"it{t}", name=f"it{t}")
            nc.sync.dma_start(it[:], idx[rows, :])
            its.append(it)
        for t in range(NT):
            es4 = ipool.tile([TILE, T * L * (M // 2)], f32, tag=f"es4_{t % 2}")
            nc.gpsimd.indirect_dma_start(
                out=es4[:],
                out_offset=None,
                in_=table[:],
                in_offset=bass.IndirectOffsetOnAxis(ap=its[t][:], axis=0),
            )
            es4s.append(es4)

        xt = wpool.tile([13, BC], bf16)
        nc.sync.dma_start(xt[:], xT[:])
        t_wb0 = wpool.tile([13, 512], bf16)
        t_wb1 = wpool.tile([128, 1024], bf16)
        t_wb2 = wpool.tile([128, 128], bf16)
        t_wt0x = wpool.tile([64, 512], bf16)
        t_wt0z = wpool.tile([128, NZCH * 512], bf16)
        t_wt1 = wpool.tile([128, 1024], bf16)
        t_wt2 = wpool.tile([128, 2], bf16)
        t_bb0 = wpool.tile([128, 4], f32)
        t_bb1 = wpool.tile([128, 2], f32)
        t_bb2 = wpool.tile([64, 1], f32)
        t_bt0 = wpool.tile([128, 4], f32)
        t_bt1 = wpool.tile([128, 2], f32)
        t_bt2 = wpool.tile([1, 1], f32)
        for t_, d_ in [(t_wb0, wb0), (t_wb1, wb1), (t_wb2, wb2), (t_wt0x, wt0x),
                       (t_wt0z, wt0z), (t_wt1, wt1), (t_wt2, wt2), (t_bb0, bb0),
                       (t_bb1, bb1), (t_bb2, bb2), (t_bt0, bt0), (t_bt1, bt1),
                       (t_bt2, bt2)]:
            nc.sync.dma_start(t_[:], d_[:])
        ident = wpool.tile([128, 128], bf16)
        make_identity(nc, ident[:])

        # sample-major cols (col = c*27 + e): gram stationary APs must be
        # contiguous (strided LDWEIGHTS is fatal on HW). Drains iterate
        # (k outer, c inner): sequential PSUM reads, stride-27 SBUF writes.
        ptf = ppool.tile([128, E27 * BC], bf16, name="ptf", tag="ptf")
        ptf_r = ptf[:].rearrange("p (s e) -> p s e", e=E27)
        ptf_e = ptf[:].rearrange("p (s e) -> p e s", e=E27)
        bxc = ppool.tile([64, BC], bf16, name="bxc", tag="bxc")
        bxp = ppool.tile([64, BC], bf16, name="bxp", tag="bxp")

        zts = []
        for ci in range(NZCH):
            zt_ = zpool.tile([128, BC], bf16, name=f"zt{ci}", tag=f"zt{ci}")
            nc.vector.memset(zt_[:], 0.0)
            zts.append(zt_)

        # --- bottom MLP, 512-wide feature-major ---
        h0 = hpool.tile([128, 4 * BC], bf16, name="h0", tag="h0")
        for ob in range(4):
            ps = mmpool.tile([128, BC], f32, tag="ps")
            nc.tensor.matmul(ps[:], lhsT=t_wb0[:, ob * 128:(ob + 1) * 128],
                             rhs=xt[:], start=True, stop=True)
            nc.scalar.activation(h0[:, ob * BC:(ob + 1) * BC], ps[:],
                                 Relu, bias=t_bb0[:, ob:ob + 1])
        h1 = hpool.tile([128, 2 * BC], bf16, name="h1", tag="h1")
        for ob in range(2):
            ps = mmpool.tile([128, BC], f32, tag="ps")
            for kc in range(4):
                nc.tensor.matmul(
                    ps[:],
                    lhsT=t_wb1[:, kc * 256 + ob * 128: kc * 256 + (ob + 1) * 128],
                    rhs=h0[:, kc * BC:(kc + 1) * BC],
                    start=(kc == 0), stop=(kc == 3))
            nc.scalar.activation(h1[:, ob * BC:(ob + 1) * BC], ps[:],
                                 Relu, bias=t_bb1[:, ob:ob + 1])
        ps = mmpool.tile([64, BC], f32, tag="ps")
        for kc in range(2):
            nc.tensor.matmul(ps[:], lhsT=t_wb2[:, kc * 64:(kc + 1) * 64],
                             rhs=h1[:, kc * BC:(kc + 1) * BC],
                             start=(kc == 0), stop=(kc == 1))
        # x -> bxc (natural order), copies into both PTflat bands' slot 26,
        # plus permuted bxp for the top-L0 rhs (matches zt column order)
        nc.scalar.activation(bxc[:], ps[:], Relu, bias=t_bb2[:, 0:1])
        nc.vector.tensor_copy(ptf_e[0:64, 26, :], bxc[:])
        nc.scalar.copy(ptf_e[64:128, 26, :], bxc[:])
        # src iterated in natural (t,s,gi) order -> dst col 128*s + 32*t + gi
        bxp_d = bxp[:].rearrange("p (s t gi) -> p t s gi", s=4, t=4)
        bxc_s = bxc[:].rearrange("p (t s gi) -> p t s gi", t=4, s=4)
        nc.vector.tensor_copy(bxp_d, bxc_s)

        # --- per-tile: pool (DVE) + transposes -> PTflat ---
        def do_tile(t):
            es4_r = es4s[t][:].bitcast(bf16).rearrange(
                "p (g k l m) -> p g k l m", g=2, k=13, l=L)
            es = epool.tile([TILE, T * M], bf16)
            es_r = es[:].rearrange("p (k g m) -> p g k m", g=2, k=13)
            nc.vector.tensor_add(es_r, es4_r[:, :, :, 0, :], es4_r[:, :, :, 1, :])
            nc.vector.tensor_add(es_r, es_r, es4_r[:, :, :, 2, :])
            nc.vector.tensor_add(es_r, es_r, es4_r[:, :, :, 3, :])

            tp1 = tppool.tile([128, 7 * TILE], bf16, tag="tp1")
            tp2 = tppool.tile([128, 6 * TILE], bf16, tag="tp2")
            for j in range(13):
                k = (j // 2) if j % 2 == 0 else (7 + j // 2)
                dst = (tp1[:, k * TILE:(k + 1) * TILE] if k < 7
                       else tp2[:, (k - 7) * TILE:(k - 6) * TILE])
                nc.tensor.transpose(dst, in_=es[:, k * 128:(k + 1) * 128],
                                    identity=ident[:])
            cols = slice(t * TILE, (t + 1) * TILE)
            s_lo1 = tp1[0:64, :].rearrange("p (k c) -> p k c", k=7)
            s_hi1 = tp1[64:128, :].rearrange("p (k c) -> p k c", k=7)
            s_lo2 = tp2[0:64, :].rearrange("p (k c) -> p k c", k=6)
            s_hi2 = tp2[64:128, :].rearrange("p (k c) -> p k c", k=6)
            drains = []
            for band in (0, 64):
                pe_ = ptf_e[band:band + 64]
                drains += [(pe_[:, 0:7, cols], s_lo1), (pe_[:, 13:20, cols], s_hi1),
                           (pe_[:, 7:13, cols], s_lo2), (pe_[:, 20:26, cols], s_hi2)]
            for i, (dd, ss) in enumerate(drains):
                if i % 2 == 0:
                    nc.vector.tensor_copy(dd, ss)
                else:
                    nc.scalar.copy(dd, ss)

        # --- grams + extraction per tile-pair (256 samples in 4 banks) ---
        prev_ext = []
        gstate = {}

        def do_gram_mms(tp):
            g = gpool.tile([128, 4 * 512], f32, tag="gram")
            mm_last = {}
            for tl in range(2):
                t = 2 * tp + tl
                for gi in range(32):
                    for strip in range(4):
                        u = 32 * strip + gi
                        col = tl * 1024 + (gi // 16) * 512 + 32 * (gi % 16)
                        band = 64 * (strip % 2)
                        sap = ptf_r[band:band + 64, 128 * t + u, :]
                        mm = nc.tensor.matmul(
                            g[32 * strip:32 * strip + E27, col:col + E27],
                            lhsT=sap, rhs=sap, start=True, stop=True,
                            tile_position=(band, 32 * strip))
                        mm_last[strip] = mm
                        for e in prev_ext:
                            add_dep_helper(mm.ins, e.ins,
                                           reason="gram MMs wait prev extraction (bank hazard)")
            gstate[tp] = (g, mm_last)

        def do_ext(tp):
            nonlocal prev_ext
            g, mm_last = gstate[tp]
            ext_last = {}
            # extraction copy (I, strip): contiguous 64-col dst at
            # 128*strip + 64*tp; src (tl, b, w) levels over 4 banks
            g_r = g[:].rearrange("p (tl b w r) -> p tl b w r", tl=2, b=2, w=16)
            for I in range(1, E27):
                ci, local = (I - 1) // 4, 32 * ((I - 1) % 4)
                zr = zts[ci][:].rearrange(
                    "p (s q tl b w o) -> p s q tl b w o", s=4, q=2, tl=2, b=2, o=1)
                for strip in range(4):
                    src = g_r[32 * strip:32 * strip + E27, :, :, :, I:I + 1]
                    dst = zr[local:local + E27, strip, tp]
                    if I % 3 != 0:
                        e = nc.vector.tensor_copy(dst, src)
                        ext_last[("v", strip)] = e
                    else:
                        e = nc.scalar.copy(dst, src)
                        ext_last[("s", strip)] = e
                    for s2 in range(4):
                        if s2 != strip:
                            add_dep_helper(e.ins, mm_last[s2].ins,
                                           reason="extraction waits all-strip grams (bank hazard)")
            prev_ext = list(ext_last.values())

        # tiles 2,3 pooling/drains are emitted before extraction 0 so the
        # FIFO Vector/Scalar queues don't head-of-line block them behind
        # extraction copies waiting on gram MMs
        do_tile(0)
        do_tile(1)
        do_gram_mms(0)
        do_tile(2)
        do_tile(3)
        do_ext(0)
        do_gram_mms(1)
        do_ext(1)

        # --- top MLP, 512-wide (permuted column order) ---
        t0 = hpool.tile([128, 4 * BC], bf16, name="t0", tag="t0")
        for ob in range(4):
            ps = mmpool.tile([128, BC], f32, tag="ps")
            nc.tensor.matmul(ps[:], lhsT=t_wt0x[:, ob * 128:(ob + 1) * 128],
                             rhs=bxp[:], start=True, stop=False)
            for zc in range(NZCH):
                nc.tensor.matmul(
                    ps[:],
                    lhsT=t_wt0z[:, zc * 512 + ob * 128: zc * 512 + (ob + 1) * 128],
                    rhs=zts[zc][:, :], start=False, stop=(zc == NZCH - 1))
            nc.scalar.activation(t0[:, ob * BC:(ob + 1) * BC], ps[:],
                                 Relu, bias=t_bt0[:, ob:ob + 1])
        t1 = hpool.tile([128, 2 * BC], bf16, name="t1", tag="t1")
        for ob in range(2):
            ps = mmpool.tile([128, BC], f32, tag="ps")
            for kc in range(4):
                nc.tensor.matmul(
                    ps[:],
                    lhsT=t_wt1[:, kc * 256 + ob * 128: kc * 256 + (ob + 1) * 128],
                    rhs=t0[:, kc * BC:(kc + 1) * BC],
                    start=(kc == 0), stop=(kc == 3))
            nc.scalar.activation(t1[:, ob * BC:(ob + 1) * BC], ps[:],
                                 Relu, bias=t_bt1[:, ob:ob + 1])
        pso = mmpool.tile([1, BC], f32, tag="ps")
        for kc in range(2):
            nc.tensor.matmul(pso[:], lhsT=t_wt2[:, kc:kc + 1],
                             rhs=t1[:, kc * BC:(kc + 1) * BC],
                             start=(kc == 0), stop=(kc == 1))
        osb = opool.tile([1, BC], f32)
        nc.scalar.activation(osb[:], pso[:], Sigmoid, bias=t_bt2[:, 0:1])
        nc.sync.dma_start(out[:], osb[:])

    nc.compile()
    return nc


def _pack_k(w):
    K, N = w.shape
    return np.ascontiguousarray(
        w.reshape(K // 128, 128, N).transpose(1, 0, 2).reshape(128, -1))


def _host_inputs(dense_x, sparse_idx, emb_tables,
                 bot_W0, bot_b0, bot_W1, bot_b1, bot_W2, bot_b2,
                 top_W0, top_b0, top_W1, top_b1, top_W2, top_b2):
    f32 = np.float32
    table_bf = np.ascontiguousarray(emb_tables.reshape(T * NR, M)).astype(_BF)
    table = table_bf.view(f32)                                       # [T*NR, 32]
    flat_idx = (np.asarray(sparse_idx, dtype=np.int64)
                + (np.arange(T, dtype=np.int64) * NR)[None, :, None]).astype(np.int32)
    idx_tl = flat_idx.reshape(B, T * L)                              # [B, 104]
    xTh = np.ascontiguousarray(np.asarray(dense_x, f32).T).astype(_BF)  # [13, B]

    # W0z rows into box layout under SLOT indexing (slot s = Tcat perm[s])
    wt0z_full = np.asarray(top_W0, f32)[:, 64:].T                     # [351, 512]
    perm = np.array(list(range(1, 27)) + [0])
    wt0z_pad = np.zeros((ZPAD, 512), f32)
    for I in range(1, E27):
        for J in range(I):
            a, b = perm[I], perm[J]
            hi, lo = (a, b) if a > b else (b, a)
            p = hi * (hi - 1) // 2 + lo
            wt0z_pad[32 * (I - 1) + J] = wt0z_full[p]

    shared = {
        "table": table,
        "wb0": np.ascontiguousarray(np.asarray(bot_W0, f32).T).astype(_BF),
        "wb1": _pack_k(np.asarray(bot_W1, f32).T).astype(_BF),
        "wb2": _pack_k(np.asarray(bot_W2, f32).T).astype(_BF),
        "wt0x": np.ascontiguousarray(np.asarray(top_W0, f32)[:, :64].T).astype(_BF),
        "wt0z": _pack_k(wt0z_pad).astype(_BF),
        "wt1": _pack_k(np.asarray(top_W1, f32).T).astype(_BF),
        "wt2": _pack_k(np.asarray(top_W2, f32).T).astype(_BF),
        "bb0": np.ascontiguousarray(np.asarray(bot_b0, f32).reshape(4, 128).T),
        "bb1": np.ascontiguousarray(np.asarray(bot_b1, f32).reshape(2, 128).T),
        "bb2": np.asarray(bot_b2, f32).reshape(64, 1).copy(),
        "bt0": np.ascontiguousarray(np.asarray(top_b0, f32).reshape(4, 128).T),
        "bt1": np.ascontiguousarray(np.asarray(top_b1, f32).reshape(2, 128).T),
        "bt2": np.asarray(top_b2, f32).reshape(1, 1).copy(),
    }
    in_maps = []
    for c in range(NCORES):
        sl = slice(c * BC, (c + 1) * BC)
        m = dict(shared)
        m["xT"] = np.ascontiguousarray(xTh[:, sl])
        m["idx"] = np.ascontiguousarray(idx_tl[sl, :])
        in_maps.append(m)
    return in_maps


_ZTCOL = None


def _unpermute(core_out):
    """kernel osb columns are in zt order; map back to natural order."""
    global _ZTCOL
    if _ZTCOL is None:
        _ZTCOL = np.array([_ztcol(c) for c in range(BC)])
    return core_out[_ZTCOL]


def kernel(**inputs):
    from concourse import bass_utils

    if "prog" not in _prog_cache:
        _prog_cache["prog"] = build_program()
    nc = _prog_cache["prog"]
    in_maps = _host_inputs(**inputs)
    res = bass_utils.run_bass_kernel_spmd(nc, in_maps, core_ids=list(range(NCORES)))
    outs = [_unpermute(r["out"].reshape(BC)).reshape(BC, 1) for r in res.results]
    return np.concatenate(outs, axis=0).astype(np.float32)


if __name__ == "__main__":
    prog = build_program()
    print("program built OK")
